# revision 55
# baseline (speedup 1.0000x reference)
"""Multi-head attention (RoPE, causal) Trainium2 Bass kernel, 8-way sharded.

Sharding: core c handles batch b = c//4 and heads 4*(c%4)..4*(c%4)+3
(B*H = 32 head-rows -> 4 per core).  QKV/out projections are
Megatron-sliced per core; per-core partial outputs (row-parallel Wo)
are summed on the host.

Problem constants (hardcoded per contract):
  B=2, S=2048, D=1024, H=16, DK=64
"""

import math

import ml_dtypes
import numpy as np

import concourse.bass as bass
import concourse.mybir as mybir
import concourse.tile as tile
from concourse import bacc
from concourse.bass_utils import run_bass_kernel_spmd

B, S, D, H, DK = 2, 2048, 1024, 16, 64
E = 256            # head dims per core (4 heads x 64)
CH = 512           # sequence chunk (matmul free dim)
NCH = S // CH      # 4
NST = S // 128     # 16 s-tiles
BF16 = mybir.dt.bfloat16
F32 = mybir.dt.float32


def _np_reference_fallback(q, k, v, mask, Wq, bq, Wk, bk, Wv, bv, Wo, bo):
    """Pure-numpy reference path (only used for inputs outside the
    contract: non-causal mask or nonzero qkv biases)."""
    qh = (q @ Wq.T + bq).reshape(B, S, H, DK)
    kh = (k @ Wk.T + bk).reshape(B, S, H, DK)
    vh = (v @ Wv.T + bv).reshape(B, S, H, DK)
    inv_freq = 1.0 / (10000.0 ** (np.arange(0, DK, 2, dtype=np.float32) / DK))
    pos = np.arange(S, dtype=np.float32)
    fr = pos[:, None] * inv_freq[None, :]
    cos, sin = np.cos(fr)[:, None, :], np.sin(fr)[:, None, :]

    def rope(x):
        t = DK // 2
        x1, x2 = x[..., :t], x[..., t:]
        return np.concatenate([x1 * cos - x2 * sin, x1 * sin + x2 * cos], -1)

    qh, kh = rope(qh), rope(kh)
    sc = np.einsum('bqhd,bkhd->bhqk', qh, kh) / math.sqrt(DK)
    sc = np.where(mask == 0, np.float32(-10000.0), sc)
    sc = sc - sc.max(-1, keepdims=True)
    e = np.exp(sc)
    attn = e / e.sum(-1, keepdims=True)
    out = np.einsum('bhqk,bkhd->bqhd', attn, vh).reshape(B, S, D)
    return (out @ Wo.T + bo).astype(np.float32)


def _build_program():
    nc = bacc.Bacc(None, target_bir_lowering=False)

    dp = nc.declare_dram_parameter
    xq = dp("xq", [D, S], BF16, isOutput=False)   # q[b].T
    xk = dp("xk", [D, S], BF16, isOutput=False)
    xv = dp("xv", [D, S], BF16, isOutput=False)
    wq = dp("wq", [D, E], BF16, isOutput=False)   # Wq_c.T
    wk = dp("wk", [D, E], BF16, isOutput=False)
    wv = dp("wv", [D, E], BF16, isOutput=False)
    wo = dp("wo", [E, D], BF16, isOutput=False)   # Wo_c.T rows
    ct = dp("ct", [128, S], BF16, isOutput=False)  # cos table (1/sqrt8 folded)
    st = dp("st", [128, S], BF16, isOutput=False)  # signed sin table
    rt = dp("rt", [128, 128], BF16, isOutput=False)  # half-swap permutation
    tri = dp("tri", [128, 896], BF16, isOutput=False)  # wide causal 0/1 ramp
    out = dp("out", [S, D], BF16, isOutput=True)
    den_d = nc.dram_tensor("den_d", [16, CH], F32)   # denominator bounce rows
    rec_d = nc.dram_tensor("rec_d", [16, CH], F32)   # reciprocal bounce rows

    with tile.TileContext(nc) as tc:
        with (
            tc.tile_pool(name="const", bufs=1) as const,
            tc.tile_pool(name="persist", bufs=1) as persist,
            tc.tile_pool(name="xt", bufs=8) as xtp,
            tc.tile_pool(name="raw", bufs=3) as rawp,
            tc.tile_pool(name="ropetmp", bufs=4) as rtp,
            tc.tile_pool(name="pblk", bufs=5) as pblk,
            tc.tile_pool(name="maskp", bufs=4) as maskp,
            tc.tile_pool(name="norm", bufs=6) as normp,
            tc.tile_pool(name="obuf", bufs=4) as obufp,
            tc.tile_pool(name="projps", bufs=2, space="PSUM") as projps,
        ):
            # ---- constants to SBUF ----
            rt_t = const.tile([128, 128], BF16, tag="rt")
            nc.gpsimd.dma_start(out=rt_t[:], in_=rt[:])
            wk_t = const.tile([128, 8, E], BF16, tag="wk")
            wk_r = wk[:].rearrange("(kt p) e -> p kt e", p=128)
            for kt in range(8):
                nc.sync.dma_start(out=wk_t[:, kt, :], in_=wk_r[:, kt, :])
            wq_t = const.tile([128, 8, E], BF16, tag="wq")
            # m=0 and m=1 head-pairs share identical RoPE tables
            ct_t = const.tile([128, S], BF16, tag="ct")
            st_t = const.tile([128, S], BF16, tag="st")
            wv_t = const.tile([128, 8, E], BF16, tag="wv")
            tri_t = const.tile([128, 896], BF16, tag="tri")
            wo_t = const.tile([128, 2, D], BF16, tag="wo")
            ones_t = const.tile([1, 64], BF16, tag="ones")
            nc.vector.memset(ones_t[:], 1.0)

            # ---- persistent intermediates ----
            qT = persist.tile([128, 2, S], BF16, tag="qT")   # partitions: e%128, dim1: e//128
            # kT split into two zero-padded copies so score matmuls can use a
            # full 128-row contraction (keeps the PE in 128x128 tiling mode —
            # no mode-switch drains between score and AV/proj matmuls).
            kTl = persist.tile([128, 2, S], BF16, tag="kTl")  # rows 64:128 zero
            kTh = persist.tile([128, 2, S], BF16, tag="kTh")  # rows 0:64 zero
            nc.gpsimd.memset(kTl[64:128, :, :], 0.0)
            nc.gpsimd.memset(kTh[0:64, :, :], 0.0)
            aT = persist.tile([128, 2, S], BF16, tag="aT")
            vext = persist.tile([128, NST, 4, 65], BF16, tag="vext")
            nc.vector.memset(vext[:, :, :, 64:65], 1.0)
            # zero the exp-output pool once so regions skipped by exp can
            # never hold NaN bit patterns (finite stale data is fine)
            for _ in range(5):
                p0 = pblk.tile([128, 2 * CH], BF16, tag="p")
                nc.vector.memset(p0[:], 0.0)

            def load_x_chunk(x_dram, c, dma_eng, split=False):
                xt = xtp.tile([128, 8, CH], BF16, tag="xt")
                xsrc = x_dram[:].rearrange("(kt p) s -> p kt s", p=128)
                if split:
                    for kt in range(8):
                        dma_eng.dma_start(out=xt[:, kt, :], in_=xsrc[:, kt, c * CH:(c + 1) * CH])
                else:
                    dma_eng.dma_start(out=xt[:], in_=xsrc[:, :, c * CH:(c + 1) * CH])
                return xt

            def proj_rope_chunk(xt, w_t, dests, c):
                """Project chunk c of q/k into [e, s] layout + rope.
                dests: list of (tile, row_lo, row_hi) output slices."""
                for m in range(2):
                    ps = projps.tile([128, CH], F32, tag="ps")
                    for kt in range(8):
                        nc.tensor.matmul(
                            ps[:], lhsT=w_t[:, kt, m * 128:(m + 1) * 128],
                            rhs=xt[:, kt, :], start=(kt == 0), stop=(kt == 7),
                        )
                    raw = rawp.tile([128, CH], BF16, tag="raw")
                    nc.scalar.copy(raw[:], ps[:])
                    rps = projps.tile([128, CH], F32, tag="ps")
                    nc.tensor.matmul(rps[:], lhsT=rt_t[:], rhs=raw[:], start=True, stop=True)
                    t1 = rtp.tile([128, CH], BF16, tag="rtmp")
                    nc.vector.tensor_mul(t1[:], rps[:], st_t[:, c * CH:(c + 1) * CH])
                    t2 = rtp.tile([128, CH], BF16, tag="rtmp")
                    nc.vector.tensor_mul(t2[:], raw[:], ct_t[:, c * CH:(c + 1) * CH])
                    for dest, r0, r1 in dests:
                        nc.vector.tensor_add(
                            dest[r0:r1, m, c * CH:(c + 1) * CH], t1[r0:r1, :], t2[r0:r1, :])

            def vproj_stile(xt_v, stl):
                """Project s-tile stl of v into vext [s, (h, e)] layout."""
                ps = projps.tile([128, E], F32, tag="ps")
                for kt in range(8):
                    nc.tensor.matmul(
                        ps[:], lhsT=xt_v[:, kt, (stl % 4) * 128:(stl % 4) * 128 + 128],
                        rhs=wv_t[:, kt, :], start=(kt == 0), stop=(kt == 7),
                    )
                nc.scalar.copy(
                    vext[:, stl, :, 0:64],
                    ps[:].rearrange("p (h e) -> p h e", h=4),
                )

            def attention_chunk(c, spair, opsum, filler=None):
                nj = 4 * c + 4
                oc_tiles = []
                for pair in range(2):
                    o_lo = opsum.tile([65, CH], F32, tag="o")
                    o_hi = opsum.tile([65, CH], F32, tag="o")
                    h_lo, h_hi = 2 * pair, 2 * pair + 1
                    # -- all sk-tile pairs; diagonal tiles masked post-exp --
                    for jj in range(0, nj, 2):
                        # leading columns that are fully masked for both tiles
                        gmin = max(0, (jj - 4 * c) * 128)
                        for half, ob, pt in ((0, o_lo, 0), (64, o_hi, 1)):
                            kT = kTl if half == 0 else kTh
                            sp = spair.tile([128, 2 * CH], F32, tag="sp")
                            for dj in range(2):
                                j = jj + dj
                                g0 = dj * CH + (gmin if dj == 0 else 0)
                                nc.tensor.matmul(
                                    sp[:, g0:(dj + 1) * CH],
                                    lhsT=kT[:, pair, j * 128:(j + 1) * 128],
                                    rhs=qT[:, pair,
                                           c * CH + (g0 - dj * CH):(c + 1) * CH],
                                    start=True, stop=True,
                                )
                            # p buffers are zeroed once at startup; the
                            # region [0:gmin] skipped by exp only ever holds
                            # finite stale values, and the tri mask zeroes it
                            # in pm before the AV matmul reads it.
                            p = pblk.tile([128, 2 * CH], BF16, tag="p")
                            nc.scalar.activation(
                                p[:, gmin:], sp[:, gmin:],
                                mybir.ActivationFunctionType.Exp)
                            for dj in range(2):
                                j = jj + dj
                                src = p
                                s0 = dj * CH
                                if j >= 4 * c:
                                    g = (j - 4 * c) * 128
                                    pm = maskp.tile([128, CH], BF16, tag="pm")
                                    nc.vector.tensor_mul(
                                        pm[:],
                                        p[:, dj * CH:(dj + 1) * CH],
                                        tri_t[:, 384 - g:896 - g],
                                    )
                                    src = pm
                                    s0 = 0
                                nc.tensor.matmul(
                                    ob[:], lhsT=vext[:, j, 2 * pair + pt, :],
                                    rhs=src[:, s0:s0 + CH],
                                    start=(j == 0), stop=(j == nj - 1),
                                    skip_group_check=True,
                                )
                    # -- evict O to SBUF (frees PSUM); stash oc tiles --
                    oc_lo = normp.tile([65, CH], F32, tag="oc")
                    nc.vector.tensor_copy(oc_lo[:], o_lo[:])
                    oc_hi = normp.tile([65, CH], F32, tag="oc")
                    nc.vector.tensor_copy(oc_hi[:], o_hi[:])
                    oc_tiles.append((pair, oc_lo, oc_hi))
                    # filler (prev chunk's Wo) goes AFTER this pair's score
                    # work in the PE queue, so its aT dependency (previous
                    # normalize bounce) has a full pair of attention to
                    # resolve under — no PE head-block at chunk start.
                    if filler is not None:
                        filler(pair)
                return oc_tiles

            def normalize_pair(c, pair, oc_lo, oc_hi):
                # pair-batched DRAM-bounce reciprocal.  The whole chain is
                # deprioritized so the Tile scheduler never slots the
                # reciprocal ahead of the next chunk's rope ops in the DVE
                # queue (the DVE is FIFO — a recip waiting on the slow
                # gather would head-block the score matmuls' dependencies).
                r0 = 4 * c + 2 * pair
                nc.gpsimd.dma_start(out=den_d[:][r0:r0 + 1, :], in_=oc_lo[64:65, :])
                nc.gpsimd.dma_start(out=den_d[:][r0 + 1:r0 + 2, :], in_=oc_hi[64:65, :])
                with tc.high_priority(offset=-400):
                    sc8t = normp.tile([16, 2, CH // 16], F32, tag="sc8t")
                    nc.gpsimd.dma_start(
                        out=sc8t[:],
                        in_=den_d[:].rearrange("r (p j) -> p r j", p=16)[:, r0:r0 + 2, :])
                    rc8t = normp.tile([16, 2, CH // 16], F32, tag="rc8t")
                    nc.vector.reciprocal(rc8t[:], sc8t[:])
                    nc.gpsimd.dma_start(
                        out=rec_d[:].rearrange("r (p j) -> p r j", p=16)[:, r0:r0 + 2, :],
                        in_=rc8t[:])
                    rsrc = rec_d[:]
                    for half, oc in ((0, oc_lo), (64, oc_hi)):
                        ridx = r0 + (half // 64)
                        rbc = normp.tile([64, CH], F32, tag="rbc")
                        nc.gpsimd.dma_start(
                            out=rbc[:],
                            in_=bass.AP(rsrc.tensor, rsrc.offset + ridx * CH, [[0, 64], [1, CH]]))
                        if half == 0:
                            nc.vector.tensor_mul(
                                aT[0:64, pair, c * CH:(c + 1) * CH], oc[0:64, :], rbc[:])
                        else:
                            t64 = normp.tile([64, CH], BF16, tag="t64")
                            nc.vector.tensor_mul(t64[:], oc[0:64, :], rbc[:])
                            nc.gpsimd.dma_start(
                                out=aT[64:128, pair, c * CH:(c + 1) * CH], in_=t64[:])

            def normalize_pair_direct(c, pair, oc_lo, oc_hi):
                # latency-critical tail: on-chip reciprocal + PE broadcast,
                # no DRAM round-trips.
                for half, oc in ((0, oc_lo), (64, oc_hi)):
                    # segmented: each 128-col reciprocal feeds a small PE
                    # broadcast matmul, keeping the PE active (HAM-warm)
                    # through the 1-lane reciprocal chain.
                    rrow = normp.tile([1, CH], BF16, tag="rrow")
                    bc = projps.tile([64, CH], F32, tag="ps")
                    for t in range(4):
                        sl = slice(t * 128, (t + 1) * 128)
                        with nc.allow_low_precision(reason="bf16 reciprocal of softmax denominators is well within tolerance"):
                            nc.vector.reciprocal(rrow[0:1, sl], oc[64:65, sl])
                        nc.tensor.matmul(bc[:, sl], lhsT=ones_t[:],
                                         rhs=rrow[0:1, sl],
                                         start=True, stop=True,
                                         skip_group_check=True)
                    if half == 0:
                        nc.vector.tensor_mul(
                            aT[0:64, pair, c * CH:(c + 1) * CH], oc[0:64, :], bc[:])
                    else:
                        t64 = normp.tile([64, CH], BF16, tag="t64")
                        nc.vector.tensor_mul(t64[:], oc[0:64, :], bc[:])
                        nc.sync.dma_start(
                            out=aT[64:128, pair, c * CH:(c + 1) * CH], in_=t64[:])

            def normalize_chunk(c, oc_tiles):
                for pair, oc_lo, oc_hi in oc_tiles:
                    if c == NCH - 1 and pair == 1:
                        normalize_pair_direct(c, pair, oc_lo, oc_hi)
                    else:
                        normalize_pair(c, pair, oc_lo, oc_hi)

            def wo_stiles(c, which=(0, 1, 2, 3)):
                for stl in [4 * c + w for w in which]:
                    for n in range(2):
                        ps = projps.tile([128, CH], F32, tag="ps")
                        for pair in range(2):
                            nc.tensor.matmul(
                                ps[:], lhsT=aT[:, pair, stl * 128:(stl + 1) * 128],
                                rhs=wo_t[:, pair, n * CH:(n + 1) * CH],
                                start=(pair == 0), stop=(pair == 1),
                            )
                        ob = obufp.tile([128, CH], BF16, tag="ob")
                        nc.vector.tensor_copy(ob[:], ps[:])
                        nc.sync.dma_start(
                            out=out[:].rearrange("(t p) n -> p t n", p=128)[:, stl, n * CH:(n + 1) * CH],
                            in_=ob[:],
                        )

            with (
                tc.tile_pool(name="spair", bufs=2, space="PSUM") as spair,
                tc.tile_pool(name="opsum", bufs=2, space="PSUM") as opsum,
            ):
                # PE warm-up: dummy matmuls on the tiny rt tile keep the HAM
                # activity window busy while the real inputs stream in.
                wps = spair.tile([128, 2 * CH], F32, tag="sp")
                for wi in range(96):
                    nc.tensor.matmul(
                        wps[:, 0:128], lhsT=rt_t[:], rhs=rt_t[:],
                        start=True, stop=True, skip_group_check=True,
                    )
                ct_r = ct[:]
                st_r = st[:]
                oc_by_chunk = {}
                pre = {}

                def prefetch(c):
                    if c < NCH and (c, 'k') not in pre:
                        pre[(c, 'k')] = load_x_chunk(xk, c, nc.sync)
                        pre[(c, 'q')] = load_x_chunk(xq, c, nc.sync)
                        pre[(c, 'v')] = load_x_chunk(xv, c, nc.sync)

                for c in range(NCH):
                    with nc.named_scope(f"proj_c{c}"):
                        if c == 0:
                            pre[(0, 'k')] = load_x_chunk(xk, 0, nc.sync, split=True)
                            nc.sync.dma_start(out=ct_t[:, 0:CH], in_=ct_r[:, 0:CH])
                            nc.sync.dma_start(out=st_t[:, 0:CH], in_=st_r[:, 0:CH])
                            wq_r = wq[:].rearrange("(kt p) e -> p kt e", p=128)
                            for kt in range(8):
                                nc.sync.dma_start(out=wq_t[:, kt, :], in_=wq_r[:, kt, :])
                            pre[(0, 'q')] = load_x_chunk(xq, 0, nc.gpsimd, split=True)
                        if c == 0:
                            proj_rope_chunk(pre[(c, 'k')], wk_t,
                                            [(kTl, 0, 64), (kTh, 64, 128)], c)
                            proj_rope_chunk(pre[(c, 'q')], wq_t, [(qT, 0, 128)], c)
                        else:
                            # q first: attention's off-diagonal score tiles
                            # only need qT of this chunk (kT is from earlier
                            # chunks), so they can start while k projects.
                            proj_rope_chunk(pre[(c, 'q')], wq_t, [(qT, 0, 128)], c)
                            proj_rope_chunk(pre[(c, 'k')], wk_t,
                                            [(kTl, 0, 64), (kTh, 64, 128)], c)
                        if c == 0:
                            nc.sync.dma_start(out=wv_t[:], in_=wv[:].rearrange("(kt p) e -> p kt e", p=128))
                            nc.sync.dma_start(out=tri_t[:], in_=tri[:])
                            nc.gpsimd.dma_start(out=wo_t[:], in_=wo[:].rearrange("(pt p) n -> p pt n", p=128))
                            nc.gpsimd.dma_start(out=ct_t[:, CH:S], in_=ct_r[:, CH:S])
                            nc.gpsimd.dma_start(out=st_t[:, CH:S], in_=st_r[:, CH:S])
                            pre[(0, 'v')] = load_x_chunk(xv, 0, nc.sync, split=True)
                        xt_v = pre[(c, 'v')]
                        # deep prefetch: keep the input stream two chunks
                        # ahead of compute on the dedicated sync queue
                        prefetch(c + 1)
                        prefetch(c + 2)
                        for stl in range(4 * c, 4 * c + 4):
                            vproj_stile(xt_v, stl)
                    # previous chunk's normalize is emitted AFTER this
                    # chunk's proj: its DVE reciprocal then sits behind the
                    # rope work in the DVE queue and can't head-block the
                    # score matmuls' dependencies.
                    if c >= 1:
                        with nc.named_scope(f"norm_c{c - 1}"):
                            normalize_chunk(c - 1, oc_by_chunk[c - 1])
                    def filler(pair, c=c):
                        if c >= 1:
                            wo_stiles(c - 1, which=(2 * pair, 2 * pair + 1))
                    with nc.named_scope(f"att_c{c}"):
                        oc_by_chunk[c] = attention_chunk(c, spair, opsum, filler)
                with nc.named_scope(f"norm_c{NCH - 1}"):
                    normalize_chunk(NCH - 1, oc_by_chunk[NCH - 1])
                with nc.named_scope("wo_c3"):
                    wo_stiles(NCH - 1)

    nc.compile()
    return nc


def _host_tables():
    inv_freq = 1.0 / (10000.0 ** (np.arange(0, DK, 2, dtype=np.float64) / DK))
    pos = np.arange(S, dtype=np.float64)
    fr = pos[:, None] * inv_freq[None, :]          # [S, 32]
    sc8 = 1.0 / math.sqrt(math.sqrt(DK))           # fold 1/sqrt(DK) as sqrt into q and k
    cosT = (np.cos(fr).T * sc8).astype(np.float32)  # [32, S]
    sinT = (np.sin(fr).T * sc8).astype(np.float32)
    C = np.zeros((128, S), np.float32)
    Sg = np.zeros((128, S), np.float32)
    for hh in range(2):
        C[hh * 64:hh * 64 + 32] = cosT
        C[hh * 64 + 32:hh * 64 + 64] = cosT
        Sg[hh * 64:hh * 64 + 32] = -sinT
        Sg[hh * 64 + 32:hh * 64 + 64] = sinT
    # half-swap permutation for two stacked heads (128 rows)
    R = np.zeros((128, 128), np.float32)
    for hh in range(2):
        for j in range(32):
            R[hh * 64 + j, hh * 64 + 32 + j] = 1.0
            R[hh * 64 + 32 + j, hh * 64 + j] = 1.0
    # TRIW[p, y] = 1 iff p <= y - 384; block with offset g uses cols [384-g, 896-g)
    y = np.arange(896)[None, :]
    p = np.arange(128)[:, None]
    TRI = (p <= y - 384).astype(np.float32)
    return C, Sg, R, TRI


_program_cache = {}


def kernel(q, k, v, mask, Wq, bq, Wk, bk, Wv, bv, Wo, bo):
    q = np.asarray(q, np.float32)
    k = np.asarray(k, np.float32)
    v = np.asarray(v, np.float32)
    mask = np.asarray(mask)
    Wq, bq = np.asarray(Wq, np.float32), np.asarray(bq, np.float32)
    Wk, bk = np.asarray(Wk, np.float32), np.asarray(bk, np.float32)
    Wv, bv = np.asarray(Wv, np.float32), np.asarray(bv, np.float32)
    Wo, bo = np.asarray(Wo, np.float32), np.asarray(bo, np.float32)

    causal = np.array_equal(
        np.asarray(mask[0, 0], np.int64), np.tril(np.ones((S, S), np.int64)))
    if not causal or np.any(bq) or np.any(bk):
        return _np_reference_fallback(q, k, v, mask, Wq, bq, Wk, bk, Wv, bv, Wo, bo)

    if "nc" not in _program_cache:
        _program_cache["nc"] = _build_program()
    nc = _program_cache["nc"]

    C, Sg, R, TRI = _host_tables()
    bf = ml_dtypes.bfloat16

    in_maps = []
    for c in range(8):
        b = c // 4
        h0 = 4 * (c % 4)
        sl = slice(h0 * DK, (h0 + 4) * DK)
        in_maps.append({
            "xq": np.ascontiguousarray(q[b].T).astype(bf),
            "xk": np.ascontiguousarray(k[b].T).astype(bf),
            "xv": np.ascontiguousarray(v[b].T).astype(bf),
            "wq": np.ascontiguousarray(Wq[sl].T).astype(bf),
            "wk": np.ascontiguousarray(Wk[sl].T).astype(bf),
            "wv": np.ascontiguousarray(Wv[sl].T).astype(bf),
            "wo": np.ascontiguousarray(Wo[:, sl].T).astype(bf),
            "ct": C.astype(bf),
            "st": Sg.astype(bf),
            "rt": R.astype(bf),
            "tri": TRI.astype(bf),
        })

    res = run_bass_kernel_spmd(nc, in_maps, core_ids=list(range(8)))

    out = np.zeros((B, S, D), np.float32)
    for c in range(8):
        out[c // 4] += res.results[c]["out"].astype(np.float32)
    # bv folds through softmax (rows sum to 1) and Wo; bo direct.
    out += (bv @ Wo.T + bo)[None, None, :]
    return out


# revision 56
# speedup vs baseline: 1.0110x; 1.0110x over previous
"""Multi-head attention (RoPE, causal) Trainium2 Bass kernel, 8-way sharded.

Sharding: core c handles batch b = c//4 and heads 4*(c%4)..4*(c%4)+3
(B*H = 32 head-rows -> 4 per core).  QKV/out projections are
Megatron-sliced per core; per-core partial outputs (row-parallel Wo)
are summed on the host.

Problem constants (hardcoded per contract):
  B=2, S=2048, D=1024, H=16, DK=64
"""

import math

import ml_dtypes
import numpy as np

import concourse.bass as bass
import concourse.mybir as mybir
import concourse.tile as tile
from concourse import bacc
from concourse.bass_utils import run_bass_kernel_spmd

B, S, D, H, DK = 2, 2048, 1024, 16, 64
E = 256            # head dims per core (4 heads x 64)
CH = 512           # sequence chunk (matmul free dim)
NCH = S // CH      # 4
NST = S // 128     # 16 s-tiles
BF16 = mybir.dt.bfloat16
F32 = mybir.dt.float32


def _np_reference_fallback(q, k, v, mask, Wq, bq, Wk, bk, Wv, bv, Wo, bo):
    """Pure-numpy reference path (only used for inputs outside the
    contract: non-causal mask or nonzero qkv biases)."""
    qh = (q @ Wq.T + bq).reshape(B, S, H, DK)
    kh = (k @ Wk.T + bk).reshape(B, S, H, DK)
    vh = (v @ Wv.T + bv).reshape(B, S, H, DK)
    inv_freq = 1.0 / (10000.0 ** (np.arange(0, DK, 2, dtype=np.float32) / DK))
    pos = np.arange(S, dtype=np.float32)
    fr = pos[:, None] * inv_freq[None, :]
    cos, sin = np.cos(fr)[:, None, :], np.sin(fr)[:, None, :]

    def rope(x):
        t = DK // 2
        x1, x2 = x[..., :t], x[..., t:]
        return np.concatenate([x1 * cos - x2 * sin, x1 * sin + x2 * cos], -1)

    qh, kh = rope(qh), rope(kh)
    sc = np.einsum('bqhd,bkhd->bhqk', qh, kh) / math.sqrt(DK)
    sc = np.where(mask == 0, np.float32(-10000.0), sc)
    sc = sc - sc.max(-1, keepdims=True)
    e = np.exp(sc)
    attn = e / e.sum(-1, keepdims=True)
    out = np.einsum('bhqk,bkhd->bqhd', attn, vh).reshape(B, S, D)
    return (out @ Wo.T + bo).astype(np.float32)


def _build_program():
    nc = bacc.Bacc(None, target_bir_lowering=False)

    dp = nc.declare_dram_parameter
    xq = dp("xq", [D, S], BF16, isOutput=False)   # q[b].T
    xk = dp("xk", [D, S], BF16, isOutput=False)
    xv = dp("xv", [D, S], BF16, isOutput=False)
    wq = dp("wq", [D, E], BF16, isOutput=False)   # Wq_c.T
    wk = dp("wk", [D, E], BF16, isOutput=False)
    wv = dp("wv", [D, E], BF16, isOutput=False)
    wo = dp("wo", [E, D], BF16, isOutput=False)   # Wo_c.T rows
    ct = dp("ct", [128, S], BF16, isOutput=False)  # cos table (1/sqrt8 folded)
    st = dp("st", [128, S], BF16, isOutput=False)  # signed sin table
    rt = dp("rt", [128, 128], BF16, isOutput=False)  # half-swap permutation
    tri = dp("tri", [128, 896], BF16, isOutput=False)  # wide causal 0/1 ramp
    out = dp("out", [S, D], BF16, isOutput=True)
    den_d = nc.dram_tensor("den_d", [16, CH], F32)   # denominator bounce rows
    rec_d = nc.dram_tensor("rec_d", [16, CH], F32)   # reciprocal bounce rows

    with tile.TileContext(nc) as tc:
        with (
            tc.tile_pool(name="const", bufs=1) as const,
            tc.tile_pool(name="persist", bufs=1) as persist,
            tc.tile_pool(name="xt", bufs=8) as xtp,
            tc.tile_pool(name="raw", bufs=3) as rawp,
            tc.tile_pool(name="ropetmp", bufs=4) as rtp,
            tc.tile_pool(name="pblk", bufs=6) as pblk,
            tc.tile_pool(name="maskp", bufs=4) as maskp,
            tc.tile_pool(name="norm", bufs=7) as normp,
            tc.tile_pool(name="obuf", bufs=6) as obufp,
            tc.tile_pool(name="projps", bufs=2, space="PSUM") as projps,
        ):
            # ---- constants to SBUF ----
            rt_t = const.tile([128, 128], BF16, tag="rt")
            nc.gpsimd.dma_start(out=rt_t[:], in_=rt[:])
            wk_t = const.tile([128, 8, E], BF16, tag="wk")
            wk_r = wk[:].rearrange("(kt p) e -> p kt e", p=128)
            for kt in range(8):
                nc.sync.dma_start(out=wk_t[:, kt, :], in_=wk_r[:, kt, :])
            wq_t = const.tile([128, 8, E], BF16, tag="wq")
            # m=0 and m=1 head-pairs share identical RoPE tables
            ct_t = const.tile([128, S], BF16, tag="ct")
            st_t = const.tile([128, S], BF16, tag="st")
            wv_t = const.tile([128, 8, E], BF16, tag="wv")
            tri_t = const.tile([128, 896], BF16, tag="tri")
            wo_t = const.tile([128, 2, D], BF16, tag="wo")
            ones_t = const.tile([1, 64], BF16, tag="ones")
            nc.vector.memset(ones_t[:], 1.0)

            # ---- persistent intermediates ----
            qT = persist.tile([128, 2, S], BF16, tag="qT")   # partitions: e%128, dim1: e//128
            # kT split into two zero-padded copies so score matmuls can use a
            # full 128-row contraction (keeps the PE in 128x128 tiling mode —
            # no mode-switch drains between score and AV/proj matmuls).
            kTl = persist.tile([128, 2, S], BF16, tag="kTl")  # rows 64:128 zero
            kTh = persist.tile([128, 2, S], BF16, tag="kTh")  # rows 0:64 zero
            nc.gpsimd.memset(kTl[64:128, :, :], 0.0)
            nc.gpsimd.memset(kTh[0:64, :, :], 0.0)
            aT = persist.tile([128, 2, S], BF16, tag="aT")
            vext = persist.tile([128, NST, 4, 65], BF16, tag="vext")
            nc.vector.memset(vext[:, :, :, 64:65], 1.0)
            # zero the exp-output pool once so regions skipped by exp can
            # never hold NaN bit patterns (finite stale data is fine)
            for _ in range(6):
                p0 = pblk.tile([128, 2 * CH], BF16, tag="p")
                nc.vector.memset(p0[:], 0.0)

            def load_x_chunk(x_dram, c, dma_eng, split=False):
                xt = xtp.tile([128, 8, CH], BF16, tag="xt")
                xsrc = x_dram[:].rearrange("(kt p) s -> p kt s", p=128)
                if split:
                    for kt in range(8):
                        dma_eng.dma_start(out=xt[:, kt, :], in_=xsrc[:, kt, c * CH:(c + 1) * CH])
                else:
                    dma_eng.dma_start(out=xt[:], in_=xsrc[:, :, c * CH:(c + 1) * CH])
                return xt

            def proj_rope_chunk(xt, w_t, dests, c):
                """Project chunk c of q/k into [e, s] layout + rope.
                dests: list of (tile, row_lo, row_hi) output slices."""
                for m in range(2):
                    ps = projps.tile([128, CH], F32, tag="ps")
                    for kt in range(8):
                        nc.tensor.matmul(
                            ps[:], lhsT=w_t[:, kt, m * 128:(m + 1) * 128],
                            rhs=xt[:, kt, :], start=(kt == 0), stop=(kt == 7),
                        )
                    raw = rawp.tile([128, CH], BF16, tag="raw")
                    nc.scalar.copy(raw[:], ps[:])
                    rps = projps.tile([128, CH], F32, tag="ps")
                    nc.tensor.matmul(rps[:], lhsT=rt_t[:], rhs=raw[:], start=True, stop=True)
                    t1 = rtp.tile([128, CH], BF16, tag="rtmp")
                    nc.vector.tensor_mul(t1[:], rps[:], st_t[:, c * CH:(c + 1) * CH])
                    t2 = rtp.tile([128, CH], BF16, tag="rtmp")
                    nc.vector.tensor_mul(t2[:], raw[:], ct_t[:, c * CH:(c + 1) * CH])
                    for dest, r0, r1 in dests:
                        nc.vector.tensor_add(
                            dest[r0:r1, m, c * CH:(c + 1) * CH], t1[r0:r1, :], t2[r0:r1, :])

            def vproj_stile(xt_v, stl):
                """Project s-tile stl of v into vext [s, (h, e)] layout."""
                ps = projps.tile([128, E], F32, tag="ps")
                for kt in range(8):
                    nc.tensor.matmul(
                        ps[:], lhsT=xt_v[:, kt, (stl % 4) * 128:(stl % 4) * 128 + 128],
                        rhs=wv_t[:, kt, :], start=(kt == 0), stop=(kt == 7),
                    )
                nc.scalar.copy(
                    vext[:, stl, :, 0:64],
                    ps[:].rearrange("p (h e) -> p h e", h=4),
                )

            def attention_chunk(c, spair, opsum, filler=None):
                nj = 4 * c + 4
                oc_tiles = []
                for pair in range(2):
                    o_lo = opsum.tile([65, CH], F32, tag="o")
                    o_hi = opsum.tile([65, CH], F32, tag="o")
                    h_lo, h_hi = 2 * pair, 2 * pair + 1
                    # -- all sk-tile pairs; diagonal tiles masked post-exp --
                    for jj in range(0, nj, 2):
                        # leading columns that are fully masked for both tiles
                        gmin = max(0, (jj - 4 * c) * 128)
                        for half, ob, pt in ((0, o_lo, 0), (64, o_hi, 1)):
                            kT = kTl if half == 0 else kTh
                            sp = spair.tile([128, 2 * CH], F32, tag="sp")
                            for dj in range(2):
                                j = jj + dj
                                g0 = dj * CH + (gmin if dj == 0 else 0)
                                nc.tensor.matmul(
                                    sp[:, g0:(dj + 1) * CH],
                                    lhsT=kT[:, pair, j * 128:(j + 1) * 128],
                                    rhs=qT[:, pair,
                                           c * CH + (g0 - dj * CH):(c + 1) * CH],
                                    start=True, stop=True,
                                )
                            # p buffers are zeroed once at startup; the
                            # region [0:gmin] skipped by exp only ever holds
                            # finite stale values, and the tri mask zeroes it
                            # in pm before the AV matmul reads it.
                            p = pblk.tile([128, 2 * CH], BF16, tag="p")
                            nc.scalar.activation(
                                p[:, gmin:], sp[:, gmin:],
                                mybir.ActivationFunctionType.Exp)
                            for dj in range(2):
                                j = jj + dj
                                src = p
                                s0 = dj * CH
                                if j >= 4 * c:
                                    g = (j - 4 * c) * 128
                                    pm = maskp.tile([128, CH], BF16, tag="pm")
                                    nc.vector.tensor_mul(
                                        pm[:],
                                        p[:, dj * CH:(dj + 1) * CH],
                                        tri_t[:, 384 - g:896 - g],
                                    )
                                    src = pm
                                    s0 = 0
                                nc.tensor.matmul(
                                    ob[:], lhsT=vext[:, j, 2 * pair + pt, :],
                                    rhs=src[:, s0:s0 + CH],
                                    start=(j == 0), stop=(j == nj - 1),
                                    skip_group_check=True,
                                )
                    # -- evict O to SBUF (frees PSUM); stash oc tiles --
                    oc_lo = normp.tile([65, CH], F32, tag="oc")
                    nc.vector.tensor_copy(oc_lo[:], o_lo[:])
                    oc_hi = normp.tile([65, CH], F32, tag="oc")
                    nc.vector.tensor_copy(oc_hi[:], o_hi[:])
                    oc_tiles.append((pair, oc_lo, oc_hi))
                    # filler (prev chunk's Wo) goes AFTER this pair's score
                    # work in the PE queue, so its aT dependency (previous
                    # normalize bounce) has a full pair of attention to
                    # resolve under — no PE head-block at chunk start.
                    if filler is not None:
                        filler(pair)
                return oc_tiles

            def normalize_pair(c, pair, oc_lo, oc_hi):
                # pair-batched DRAM-bounce reciprocal.  The whole chain is
                # deprioritized so the Tile scheduler never slots the
                # reciprocal ahead of the next chunk's rope ops in the DVE
                # queue (the DVE is FIFO — a recip waiting on the slow
                # gather would head-block the score matmuls' dependencies).
                r0 = 4 * c + 2 * pair
                nc.gpsimd.dma_start(out=den_d[:][r0:r0 + 1, :], in_=oc_lo[64:65, :])
                nc.gpsimd.dma_start(out=den_d[:][r0 + 1:r0 + 2, :], in_=oc_hi[64:65, :])
                with tc.high_priority(offset=-400):
                    sc8t = normp.tile([16, 2, CH // 16], F32, tag="sc8t")
                    nc.gpsimd.dma_start(
                        out=sc8t[:],
                        in_=den_d[:].rearrange("r (p j) -> p r j", p=16)[:, r0:r0 + 2, :])
                    rc8t = normp.tile([16, 2, CH // 16], F32, tag="rc8t")
                    nc.vector.reciprocal(rc8t[:], sc8t[:])
                    nc.gpsimd.dma_start(
                        out=rec_d[:].rearrange("r (p j) -> p r j", p=16)[:, r0:r0 + 2, :],
                        in_=rc8t[:])
                    rsrc = rec_d[:]
                    for half, oc in ((0, oc_lo), (64, oc_hi)):
                        ridx = r0 + (half // 64)
                        rbc = normp.tile([64, CH], F32, tag="rbc")
                        nc.gpsimd.dma_start(
                            out=rbc[:],
                            in_=bass.AP(rsrc.tensor, rsrc.offset + ridx * CH, [[0, 64], [1, CH]]))
                        if half == 0:
                            nc.vector.tensor_mul(
                                aT[0:64, pair, c * CH:(c + 1) * CH], oc[0:64, :], rbc[:])
                        else:
                            t64 = normp.tile([64, CH], BF16, tag="t64")
                            nc.vector.tensor_mul(t64[:], oc[0:64, :], rbc[:])
                            nc.gpsimd.dma_start(
                                out=aT[64:128, pair, c * CH:(c + 1) * CH], in_=t64[:])

            def normalize_pair_direct(c, pair, oc_lo, oc_hi):
                # latency-critical tail: on-chip reciprocal + PE broadcast,
                # no DRAM round-trips.
                for half, oc in ((0, oc_lo), (64, oc_hi)):
                    # segmented: each 128-col reciprocal feeds a small PE
                    # broadcast matmul, keeping the PE active (HAM-warm)
                    # through the 1-lane reciprocal chain.
                    rrow = normp.tile([1, CH], BF16, tag="rrow")
                    bc = projps.tile([64, CH], F32, tag="ps")
                    for t in range(4):
                        sl = slice(t * 128, (t + 1) * 128)
                        with nc.allow_low_precision(reason="bf16 reciprocal of softmax denominators is well within tolerance"):
                            nc.vector.reciprocal(rrow[0:1, sl], oc[64:65, sl])
                        nc.tensor.matmul(bc[:, sl], lhsT=ones_t[:],
                                         rhs=rrow[0:1, sl],
                                         start=True, stop=True,
                                         skip_group_check=True)
                    if half == 0:
                        nc.vector.tensor_mul(
                            aT[0:64, pair, c * CH:(c + 1) * CH], oc[0:64, :], bc[:])
                    else:
                        t64 = normp.tile([64, CH], BF16, tag="t64")
                        nc.vector.tensor_mul(t64[:], oc[0:64, :], bc[:])
                        nc.sync.dma_start(
                            out=aT[64:128, pair, c * CH:(c + 1) * CH], in_=t64[:])

            def normalize_chunk(c, oc_tiles):
                for pair, oc_lo, oc_hi in oc_tiles:
                    if c == NCH - 1 and pair == 1:
                        normalize_pair_direct(c, pair, oc_lo, oc_hi)
                    else:
                        normalize_pair(c, pair, oc_lo, oc_hi)

            def wo_stiles(c, which=(0, 1, 2, 3)):
                for stl in [4 * c + w for w in which]:
                    for n in range(2):
                        ps = projps.tile([128, CH], F32, tag="ps")
                        for pair in range(2):
                            nc.tensor.matmul(
                                ps[:], lhsT=aT[:, pair, stl * 128:(stl + 1) * 128],
                                rhs=wo_t[:, pair, n * CH:(n + 1) * CH],
                                start=(pair == 0), stop=(pair == 1),
                            )
                        ob = obufp.tile([128, CH], BF16, tag="ob")
                        nc.vector.tensor_copy(ob[:], ps[:])
                        nc.sync.dma_start(
                            out=out[:].rearrange("(t p) n -> p t n", p=128)[:, stl, n * CH:(n + 1) * CH],
                            in_=ob[:],
                        )

            with (
                tc.tile_pool(name="spair", bufs=2, space="PSUM") as spair,
                tc.tile_pool(name="opsum", bufs=2, space="PSUM") as opsum,
            ):
                # PE warm-up: dummy matmuls on the tiny rt tile keep the HAM
                # activity window busy while the real inputs stream in.
                wps = spair.tile([128, 2 * CH], F32, tag="sp")
                for wi in range(120):
                    nc.tensor.matmul(
                        wps[:, 0:128], lhsT=rt_t[:], rhs=rt_t[:],
                        start=True, stop=True, skip_group_check=True,
                    )
                ct_r = ct[:]
                st_r = st[:]
                oc_by_chunk = {}
                pre = {}

                def prefetch(c):
                    if c < NCH and (c, 'k') not in pre:
                        pre[(c, 'k')] = load_x_chunk(xk, c, nc.sync)
                        pre[(c, 'q')] = load_x_chunk(xq, c, nc.sync)
                        pre[(c, 'v')] = load_x_chunk(xv, c, nc.sync)

                for c in range(NCH):
                    with nc.named_scope(f"proj_c{c}"):
                        if c == 0:
                            pre[(0, 'k')] = load_x_chunk(xk, 0, nc.sync, split=True)
                            nc.sync.dma_start(out=ct_t[:, 0:CH], in_=ct_r[:, 0:CH])
                            nc.sync.dma_start(out=st_t[:, 0:CH], in_=st_r[:, 0:CH])
                            wq_r = wq[:].rearrange("(kt p) e -> p kt e", p=128)
                            for kt in range(8):
                                nc.sync.dma_start(out=wq_t[:, kt, :], in_=wq_r[:, kt, :])
                            pre[(0, 'q')] = load_x_chunk(xq, 0, nc.gpsimd, split=True)
                        if c == 0:
                            proj_rope_chunk(pre[(c, 'k')], wk_t,
                                            [(kTl, 0, 64), (kTh, 64, 128)], c)
                            proj_rope_chunk(pre[(c, 'q')], wq_t, [(qT, 0, 128)], c)
                        else:
                            # q first: attention's off-diagonal score tiles
                            # only need qT of this chunk (kT is from earlier
                            # chunks), so they can start while k projects.
                            proj_rope_chunk(pre[(c, 'q')], wq_t, [(qT, 0, 128)], c)
                            proj_rope_chunk(pre[(c, 'k')], wk_t,
                                            [(kTl, 0, 64), (kTh, 64, 128)], c)
                        if c == 0:
                            nc.sync.dma_start(out=wv_t[:], in_=wv[:].rearrange("(kt p) e -> p kt e", p=128))
                            nc.sync.dma_start(out=tri_t[:], in_=tri[:])
                            nc.gpsimd.dma_start(out=wo_t[:], in_=wo[:].rearrange("(pt p) n -> p pt n", p=128))
                            nc.gpsimd.dma_start(out=ct_t[:, CH:S], in_=ct_r[:, CH:S])
                            nc.gpsimd.dma_start(out=st_t[:, CH:S], in_=st_r[:, CH:S])
                            pre[(0, 'v')] = load_x_chunk(xv, 0, nc.sync, split=True)
                        xt_v = pre[(c, 'v')]
                        # deep prefetch: keep the input stream two chunks
                        # ahead of compute on the dedicated sync queue
                        prefetch(c + 1)
                        prefetch(c + 2)
                        for stl in range(4 * c, 4 * c + 4):
                            vproj_stile(xt_v, stl)
                    # previous chunk's normalize is emitted AFTER this
                    # chunk's proj: its DVE reciprocal then sits behind the
                    # rope work in the DVE queue and can't head-block the
                    # score matmuls' dependencies.
                    if c >= 1:
                        with nc.named_scope(f"norm_c{c - 1}"):
                            normalize_chunk(c - 1, oc_by_chunk[c - 1])
                    def filler(pair, c=c):
                        if c >= 1:
                            wo_stiles(c - 1, which=(2 * pair, 2 * pair + 1))
                    with nc.named_scope(f"att_c{c}"):
                        oc_by_chunk[c] = attention_chunk(c, spair, opsum, filler)
                with nc.named_scope(f"norm_c{NCH - 1}"):
                    normalize_chunk(NCH - 1, oc_by_chunk[NCH - 1])
                with nc.named_scope("wo_c3"):
                    wo_stiles(NCH - 1)

    nc.compile()
    return nc


def _host_tables():
    inv_freq = 1.0 / (10000.0 ** (np.arange(0, DK, 2, dtype=np.float64) / DK))
    pos = np.arange(S, dtype=np.float64)
    fr = pos[:, None] * inv_freq[None, :]          # [S, 32]
    sc8 = 1.0 / math.sqrt(math.sqrt(DK))           # fold 1/sqrt(DK) as sqrt into q and k
    cosT = (np.cos(fr).T * sc8).astype(np.float32)  # [32, S]
    sinT = (np.sin(fr).T * sc8).astype(np.float32)
    C = np.zeros((128, S), np.float32)
    Sg = np.zeros((128, S), np.float32)
    for hh in range(2):
        C[hh * 64:hh * 64 + 32] = cosT
        C[hh * 64 + 32:hh * 64 + 64] = cosT
        Sg[hh * 64:hh * 64 + 32] = -sinT
        Sg[hh * 64 + 32:hh * 64 + 64] = sinT
    # half-swap permutation for two stacked heads (128 rows)
    R = np.zeros((128, 128), np.float32)
    for hh in range(2):
        for j in range(32):
            R[hh * 64 + j, hh * 64 + 32 + j] = 1.0
            R[hh * 64 + 32 + j, hh * 64 + j] = 1.0
    # TRIW[p, y] = 1 iff p <= y - 384; block with offset g uses cols [384-g, 896-g)
    y = np.arange(896)[None, :]
    p = np.arange(128)[:, None]
    TRI = (p <= y - 384).astype(np.float32)
    return C, Sg, R, TRI


_program_cache = {}


def kernel(q, k, v, mask, Wq, bq, Wk, bk, Wv, bv, Wo, bo):
    q = np.asarray(q, np.float32)
    k = np.asarray(k, np.float32)
    v = np.asarray(v, np.float32)
    mask = np.asarray(mask)
    Wq, bq = np.asarray(Wq, np.float32), np.asarray(bq, np.float32)
    Wk, bk = np.asarray(Wk, np.float32), np.asarray(bk, np.float32)
    Wv, bv = np.asarray(Wv, np.float32), np.asarray(bv, np.float32)
    Wo, bo = np.asarray(Wo, np.float32), np.asarray(bo, np.float32)

    causal = np.array_equal(
        np.asarray(mask[0, 0], np.int64), np.tril(np.ones((S, S), np.int64)))
    if not causal or np.any(bq) or np.any(bk):
        return _np_reference_fallback(q, k, v, mask, Wq, bq, Wk, bk, Wv, bv, Wo, bo)

    if "nc" not in _program_cache:
        _program_cache["nc"] = _build_program()
    nc = _program_cache["nc"]

    C, Sg, R, TRI = _host_tables()
    bf = ml_dtypes.bfloat16

    in_maps = []
    for c in range(8):
        b = c // 4
        h0 = 4 * (c % 4)
        sl = slice(h0 * DK, (h0 + 4) * DK)
        in_maps.append({
            "xq": np.ascontiguousarray(q[b].T).astype(bf),
            "xk": np.ascontiguousarray(k[b].T).astype(bf),
            "xv": np.ascontiguousarray(v[b].T).astype(bf),
            "wq": np.ascontiguousarray(Wq[sl].T).astype(bf),
            "wk": np.ascontiguousarray(Wk[sl].T).astype(bf),
            "wv": np.ascontiguousarray(Wv[sl].T).astype(bf),
            "wo": np.ascontiguousarray(Wo[:, sl].T).astype(bf),
            "ct": C.astype(bf),
            "st": Sg.astype(bf),
            "rt": R.astype(bf),
            "tri": TRI.astype(bf),
        })

    res = run_bass_kernel_spmd(nc, in_maps, core_ids=list(range(8)))

    out = np.zeros((B, S, D), np.float32)
    for c in range(8):
        out[c // 4] += res.results[c]["out"].astype(np.float32)
    # bv folds through softmax (rows sum to 1) and Wo; bo direct.
    out += (bv @ Wo.T + bo)[None, None, :]
    return out


# revision 58
# speedup vs baseline: 1.0200x; 1.0089x over previous
"""Multi-head attention (RoPE, causal) Trainium2 Bass kernel, 8-way sharded.

Sharding: core c handles batch b = c//4 and heads 4*(c%4)..4*(c%4)+3
(B*H = 32 head-rows -> 4 per core).  QKV/out projections are
Megatron-sliced per core; per-core partial outputs (row-parallel Wo)
are summed on the host.

Problem constants (hardcoded per contract):
  B=2, S=2048, D=1024, H=16, DK=64
"""

import math

import ml_dtypes
import numpy as np

import concourse.bass as bass
import concourse.mybir as mybir
import concourse.tile as tile
from concourse import bacc
from concourse.bass_utils import run_bass_kernel_spmd

B, S, D, H, DK = 2, 2048, 1024, 16, 64
E = 256            # head dims per core (4 heads x 64)
CH = 512           # sequence chunk (matmul free dim)
NCH = S // CH      # 4
NST = S // 128     # 16 s-tiles
BF16 = mybir.dt.bfloat16
F32 = mybir.dt.float32


def _np_reference_fallback(q, k, v, mask, Wq, bq, Wk, bk, Wv, bv, Wo, bo):
    """Pure-numpy reference path (only used for inputs outside the
    contract: non-causal mask or nonzero qkv biases)."""
    qh = (q @ Wq.T + bq).reshape(B, S, H, DK)
    kh = (k @ Wk.T + bk).reshape(B, S, H, DK)
    vh = (v @ Wv.T + bv).reshape(B, S, H, DK)
    inv_freq = 1.0 / (10000.0 ** (np.arange(0, DK, 2, dtype=np.float32) / DK))
    pos = np.arange(S, dtype=np.float32)
    fr = pos[:, None] * inv_freq[None, :]
    cos, sin = np.cos(fr)[:, None, :], np.sin(fr)[:, None, :]

    def rope(x):
        t = DK // 2
        x1, x2 = x[..., :t], x[..., t:]
        return np.concatenate([x1 * cos - x2 * sin, x1 * sin + x2 * cos], -1)

    qh, kh = rope(qh), rope(kh)
    sc = np.einsum('bqhd,bkhd->bhqk', qh, kh) / math.sqrt(DK)
    sc = np.where(mask == 0, np.float32(-10000.0), sc)
    sc = sc - sc.max(-1, keepdims=True)
    e = np.exp(sc)
    attn = e / e.sum(-1, keepdims=True)
    out = np.einsum('bhqk,bkhd->bqhd', attn, vh).reshape(B, S, D)
    return (out @ Wo.T + bo).astype(np.float32)


def _build_program():
    nc = bacc.Bacc(None, target_bir_lowering=False)

    dp = nc.declare_dram_parameter
    xq = dp("xq", [D, S], BF16, isOutput=False)   # q[b].T
    xk = dp("xk", [D, S], BF16, isOutput=False)
    xv = dp("xv", [D, S], BF16, isOutput=False)
    wq = dp("wq", [D, E], BF16, isOutput=False)   # Wq_c.T
    wk = dp("wk", [D, E], BF16, isOutput=False)
    wv = dp("wv", [D, E], BF16, isOutput=False)
    wo = dp("wo", [E, D], BF16, isOutput=False)   # Wo_c.T rows
    ct = dp("ct", [128, S], BF16, isOutput=False)  # cos table (1/sqrt8 folded)
    st = dp("st", [128, S], BF16, isOutput=False)  # signed sin table
    rt = dp("rt", [128, 128], BF16, isOutput=False)  # half-swap permutation
    tri = dp("tri", [128, 896], BF16, isOutput=False)  # wide causal 0/1 ramp
    out = dp("out", [S, D], BF16, isOutput=True)
    den_d = nc.dram_tensor("den_d", [16, CH], F32)   # denominator bounce rows
    rec_d = nc.dram_tensor("rec_d", [16, CH], F32)   # reciprocal bounce rows

    with tile.TileContext(nc) as tc:
        with (
            tc.tile_pool(name="const", bufs=1) as const,
            tc.tile_pool(name="persist", bufs=1) as persist,
            tc.tile_pool(name="xt", bufs=8) as xtp,
            tc.tile_pool(name="raw", bufs=3) as rawp,
            tc.tile_pool(name="ropetmp", bufs=4) as rtp,
            tc.tile_pool(name="pblk", bufs=5) as pblk,
            tc.tile_pool(name="maskp", bufs=4) as maskp,
            tc.tile_pool(name="norm", bufs=6) as normp,
            tc.tile_pool(name="obuf", bufs=4) as obufp,
            tc.tile_pool(name="projps", bufs=2, space="PSUM") as projps,
        ):
            # ---- constants to SBUF ----
            rt_t = const.tile([128, 128], BF16, tag="rt")
            nc.gpsimd.dma_start(out=rt_t[:], in_=rt[:])
            wk_t = const.tile([128, 8, E], BF16, tag="wk")
            wk_r = wk[:].rearrange("(kt p) e -> p kt e", p=128)
            for kt in range(8):
                nc.sync.dma_start(out=wk_t[:, kt, :], in_=wk_r[:, kt, :])
            wq_t = const.tile([128, 8, E], BF16, tag="wq")
            # m=0 and m=1 head-pairs share identical RoPE tables
            ct_t = const.tile([128, S], BF16, tag="ct")
            st_t = const.tile([128, S], BF16, tag="st")
            wv_t = const.tile([128, 8, E], BF16, tag="wv")
            tri_t = const.tile([128, 896], BF16, tag="tri")
            wo_t = const.tile([128, 2, D], BF16, tag="wo")
            ones_t = const.tile([1, 64], BF16, tag="ones")
            nc.vector.memset(ones_t[:], 1.0)

            # ---- persistent intermediates ----
            qT = persist.tile([128, 2, S], BF16, tag="qT")   # partitions: e%128, dim1: e//128
            # kT split into two zero-padded copies so score matmuls can use a
            # full 128-row contraction (keeps the PE in 128x128 tiling mode —
            # no mode-switch drains between score and AV/proj matmuls).
            kTl = persist.tile([128, 2, S], BF16, tag="kTl")  # rows 64:128 zero
            kTh = persist.tile([128, 2, S], BF16, tag="kTh")  # rows 0:64 zero
            nc.gpsimd.memset(kTl[64:128, :, :], 0.0)
            nc.gpsimd.memset(kTh[0:64, :, :], 0.0)
            aT = persist.tile([128, 2, S], BF16, tag="aT")
            vext = persist.tile([128, NST, 4, 65], BF16, tag="vext")
            nc.vector.memset(vext[:, :, :, 64:65], 1.0)
            # zero the exp-output pool once so regions skipped by exp can
            # never hold NaN bit patterns (finite stale data is fine)
            for _ in range(5):
                p0 = pblk.tile([128, 2 * CH], BF16, tag="p")
                nc.vector.memset(p0[:], 0.0)

            def load_x_chunk(x_dram, c, dma_eng, split=False):
                xt = xtp.tile([128, 8, CH], BF16, tag="xt")
                xsrc = x_dram[:].rearrange("(kt p) s -> p kt s", p=128)
                if split:
                    for kt in range(8):
                        dma_eng.dma_start(out=xt[:, kt, :], in_=xsrc[:, kt, c * CH:(c + 1) * CH])
                else:
                    dma_eng.dma_start(out=xt[:], in_=xsrc[:, :, c * CH:(c + 1) * CH])
                return xt

            def proj_rope_chunk(xt, w_t, dests, c):
                """Project chunk c of q/k into [e, s] layout + rope.
                dests: list of (tile, row_lo, row_hi) output slices."""
                for m in range(2):
                    ps = projps.tile([128, CH], F32, tag="ps")
                    for kt in range(8):
                        nc.tensor.matmul(
                            ps[:], lhsT=w_t[:, kt, m * 128:(m + 1) * 128],
                            rhs=xt[:, kt, :], start=(kt == 0), stop=(kt == 7),
                        )
                    raw = rawp.tile([128, CH], BF16, tag="raw")
                    nc.scalar.copy(raw[:], ps[:])
                    rps = projps.tile([128, CH], F32, tag="ps")
                    nc.tensor.matmul(rps[:], lhsT=rt_t[:], rhs=raw[:], start=True, stop=True)
                    t1 = rtp.tile([128, CH], BF16, tag="rtmp")
                    nc.vector.tensor_mul(t1[:], rps[:], st_t[:, c * CH:(c + 1) * CH])
                    t2 = rtp.tile([128, CH], BF16, tag="rtmp")
                    nc.vector.tensor_mul(t2[:], raw[:], ct_t[:, c * CH:(c + 1) * CH])
                    for dest, r0, r1 in dests:
                        nc.vector.tensor_add(
                            dest[r0:r1, m, c * CH:(c + 1) * CH], t1[r0:r1, :], t2[r0:r1, :])

            def vproj_stile(xt_v, stl):
                """Project s-tile stl of v into vext [s, (h, e)] layout."""
                ps = projps.tile([128, E], F32, tag="ps")
                for kt in range(8):
                    nc.tensor.matmul(
                        ps[:], lhsT=xt_v[:, kt, (stl % 4) * 128:(stl % 4) * 128 + 128],
                        rhs=wv_t[:, kt, :], start=(kt == 0), stop=(kt == 7),
                    )
                nc.scalar.copy(
                    vext[:, stl, :, 0:64],
                    ps[:].rearrange("p (h e) -> p h e", h=4),
                )

            def attention_chunk(c, spair, opsum, filler=None):
                nj = 4 * c + 4
                oc_tiles = []
                for pair in range(2):
                    o_lo = opsum.tile([65, CH], F32, tag="o")
                    o_hi = opsum.tile([65, CH], F32, tag="o")
                    h_lo, h_hi = 2 * pair, 2 * pair + 1
                    # -- all sk-tile pairs; diagonal tiles masked post-exp --
                    for jj in range(0, nj, 2):
                        # leading columns that are fully masked for both tiles
                        gmin = max(0, (jj - 4 * c) * 128)
                        for half, ob, pt in ((0, o_lo, 0), (64, o_hi, 1)):
                            kT = kTl if half == 0 else kTh
                            sp = spair.tile([128, 2 * CH], F32, tag="sp")
                            for dj in range(2):
                                j = jj + dj
                                g0 = dj * CH + (gmin if dj == 0 else 0)
                                nc.tensor.matmul(
                                    sp[:, g0:(dj + 1) * CH],
                                    lhsT=kT[:, pair, j * 128:(j + 1) * 128],
                                    rhs=qT[:, pair,
                                           c * CH + (g0 - dj * CH):(c + 1) * CH],
                                    start=True, stop=True,
                                )
                            # p buffers are zeroed once at startup; the
                            # region [0:gmin] skipped by exp only ever holds
                            # finite stale values, and the tri mask zeroes it
                            # in pm before the AV matmul reads it.
                            p = pblk.tile([128, 2 * CH], BF16, tag="p")
                            nc.scalar.activation(
                                p[:, gmin:], sp[:, gmin:],
                                mybir.ActivationFunctionType.Exp)
                            for dj in range(2):
                                j = jj + dj
                                src = p
                                s0 = dj * CH
                                if j >= 4 * c:
                                    g = (j - 4 * c) * 128
                                    pm = maskp.tile([128, CH], BF16, tag="pm")
                                    nc.vector.tensor_mul(
                                        pm[:],
                                        p[:, dj * CH:(dj + 1) * CH],
                                        tri_t[:, 384 - g:896 - g],
                                    )
                                    src = pm
                                    s0 = 0
                                nc.tensor.matmul(
                                    ob[:], lhsT=vext[:, j, 2 * pair + pt, :],
                                    rhs=src[:, s0:s0 + CH],
                                    start=(j == 0), stop=(j == nj - 1),
                                    skip_group_check=True,
                                )
                    # -- evict O to SBUF (frees PSUM); stash oc tiles --
                    oc_lo = normp.tile([65, CH], F32, tag="oc")
                    nc.vector.tensor_copy(oc_lo[:], o_lo[:])
                    oc_hi = normp.tile([65, CH], F32, tag="oc")
                    nc.vector.tensor_copy(oc_hi[:], o_hi[:])
                    oc_tiles.append((pair, oc_lo, oc_hi))
                    # filler (prev chunk's Wo) goes AFTER this pair's score
                    # work in the PE queue, so its aT dependency (previous
                    # normalize bounce) has a full pair of attention to
                    # resolve under — no PE head-block at chunk start.
                    if filler is not None:
                        filler(pair)
                return oc_tiles

            def normalize_pair(c, pair, oc_lo, oc_hi):
                # pair-batched DRAM-bounce reciprocal.  The whole chain is
                # deprioritized so the Tile scheduler never slots the
                # reciprocal ahead of the next chunk's rope ops in the DVE
                # queue (the DVE is FIFO — a recip waiting on the slow
                # gather would head-block the score matmuls' dependencies).
                r0 = 4 * c + 2 * pair
                nc.gpsimd.dma_start(out=den_d[:][r0:r0 + 1, :], in_=oc_lo[64:65, :])
                nc.gpsimd.dma_start(out=den_d[:][r0 + 1:r0 + 2, :], in_=oc_hi[64:65, :])
                with tc.high_priority(offset=-400):
                    sc8t = normp.tile([16, 2, CH // 16], F32, tag="sc8t")
                    nc.gpsimd.dma_start(
                        out=sc8t[:],
                        in_=den_d[:].rearrange("r (p j) -> p r j", p=16)[:, r0:r0 + 2, :])
                    rc8t = normp.tile([16, 2, CH // 16], F32, tag="rc8t")
                    nc.vector.reciprocal(rc8t[:], sc8t[:])
                    nc.gpsimd.dma_start(
                        out=rec_d[:].rearrange("r (p j) -> p r j", p=16)[:, r0:r0 + 2, :],
                        in_=rc8t[:])
                    rsrc = rec_d[:]
                    for half, oc in ((0, oc_lo), (64, oc_hi)):
                        ridx = r0 + (half // 64)
                        rbc = normp.tile([64, CH], F32, tag="rbc")
                        nc.gpsimd.dma_start(
                            out=rbc[:],
                            in_=bass.AP(rsrc.tensor, rsrc.offset + ridx * CH, [[0, 64], [1, CH]]))
                        if half == 0:
                            nc.vector.tensor_mul(
                                aT[0:64, pair, c * CH:(c + 1) * CH], oc[0:64, :], rbc[:])
                        else:
                            t64 = normp.tile([64, CH], BF16, tag="t64")
                            nc.vector.tensor_mul(t64[:], oc[0:64, :], rbc[:])
                            nc.gpsimd.dma_start(
                                out=aT[64:128, pair, c * CH:(c + 1) * CH], in_=t64[:])

            def normalize_pair_direct(c, pair, oc_lo, oc_hi):
                # latency-critical tail: on-chip reciprocal + PE broadcast,
                # no DRAM round-trips.
                for half, oc in ((0, oc_lo), (64, oc_hi)):
                    # segmented: each 128-col reciprocal feeds a small PE
                    # broadcast matmul, keeping the PE active (HAM-warm)
                    # through the 1-lane reciprocal chain.
                    rrow = normp.tile([1, CH], BF16, tag="rrow")
                    bc = projps.tile([64, CH], F32, tag="ps")
                    for t in range(4):
                        sl = slice(t * 128, (t + 1) * 128)
                        with nc.allow_low_precision(reason="bf16 reciprocal of softmax denominators is well within tolerance"):
                            nc.vector.reciprocal(rrow[0:1, sl], oc[64:65, sl])
                        nc.tensor.matmul(bc[:, sl], lhsT=ones_t[:],
                                         rhs=rrow[0:1, sl],
                                         start=True, stop=True,
                                         skip_group_check=True)
                    if half == 0:
                        nc.vector.tensor_mul(
                            aT[0:64, pair, c * CH:(c + 1) * CH], oc[0:64, :], bc[:])
                    else:
                        t64 = normp.tile([64, CH], BF16, tag="t64")
                        nc.vector.tensor_mul(t64[:], oc[0:64, :], bc[:])
                        nc.sync.dma_start(
                            out=aT[64:128, pair, c * CH:(c + 1) * CH], in_=t64[:])

            def normalize_chunk(c, oc_tiles):
                for pair, oc_lo, oc_hi in oc_tiles:
                    if c == NCH - 1 and pair == 1:
                        normalize_pair_direct(c, pair, oc_lo, oc_hi)
                    else:
                        normalize_pair(c, pair, oc_lo, oc_hi)

            def wo_stiles(c, which=(0, 1, 2, 3)):
                for stl in [4 * c + w for w in which]:
                    for n in range(2):
                        ps = projps.tile([128, CH], F32, tag="ps")
                        for pair in range(2):
                            nc.tensor.matmul(
                                ps[:], lhsT=aT[:, pair, stl * 128:(stl + 1) * 128],
                                rhs=wo_t[:, pair, n * CH:(n + 1) * CH],
                                start=(pair == 0), stop=(pair == 1),
                            )
                        ob = obufp.tile([128, CH], BF16, tag="ob")
                        nc.vector.tensor_copy(ob[:], ps[:])
                        nc.sync.dma_start(
                            out=out[:].rearrange("(t p) n -> p t n", p=128)[:, stl, n * CH:(n + 1) * CH],
                            in_=ob[:],
                        )

            with (
                tc.tile_pool(name="spair", bufs=2, space="PSUM") as spair,
                tc.tile_pool(name="opsum", bufs=2, space="PSUM") as opsum,
            ):
                # PE warm-up: dummy matmuls on the tiny rt tile keep the HAM
                # activity window busy while the real inputs stream in.
                wps = spair.tile([128, 2 * CH], F32, tag="sp")
                for wi in range(96):
                    nc.tensor.matmul(
                        wps[:, 0:128], lhsT=rt_t[:], rhs=rt_t[:],
                        start=True, stop=True, skip_group_check=True,
                    )
                ct_r = ct[:]
                st_r = st[:]
                oc_by_chunk = {}
                pre = {}

                def prefetch(c):
                    if c < NCH and (c, 'k') not in pre:
                        pre[(c, 'k')] = load_x_chunk(xk, c, nc.sync)
                        pre[(c, 'q')] = load_x_chunk(xq, c, nc.sync)
                        pre[(c, 'v')] = load_x_chunk(xv, c, nc.sync)

                for c in range(NCH):
                    with nc.named_scope(f"proj_c{c}"):
                        if c == 0:
                            pre[(0, 'k')] = load_x_chunk(xk, 0, nc.sync, split=True)
                            nc.sync.dma_start(out=ct_t[:, 0:CH], in_=ct_r[:, 0:CH])
                            nc.sync.dma_start(out=st_t[:, 0:CH], in_=st_r[:, 0:CH])
                            wq_r = wq[:].rearrange("(kt p) e -> p kt e", p=128)
                            for kt in range(8):
                                nc.sync.dma_start(out=wq_t[:, kt, :], in_=wq_r[:, kt, :])
                            pre[(0, 'q')] = load_x_chunk(xq, 0, nc.gpsimd, split=True)
                        if c == 0:
                            proj_rope_chunk(pre[(c, 'k')], wk_t,
                                            [(kTl, 0, 64), (kTh, 64, 128)], c)
                            proj_rope_chunk(pre[(c, 'q')], wq_t, [(qT, 0, 128)], c)
                        else:
                            # q first: attention's off-diagonal score tiles
                            # only need qT of this chunk (kT is from earlier
                            # chunks), so they can start while k projects.
                            proj_rope_chunk(pre[(c, 'q')], wq_t, [(qT, 0, 128)], c)
                            proj_rope_chunk(pre[(c, 'k')], wk_t,
                                            [(kTl, 0, 64), (kTh, 64, 128)], c)
                        if c == 0:
                            nc.sync.dma_start(out=wv_t[:], in_=wv[:].rearrange("(kt p) e -> p kt e", p=128))
                            nc.sync.dma_start(out=tri_t[:], in_=tri[:])
                            nc.gpsimd.dma_start(out=wo_t[:], in_=wo[:].rearrange("(pt p) n -> p pt n", p=128))
                            nc.gpsimd.dma_start(out=ct_t[:, CH:S], in_=ct_r[:, CH:S])
                            nc.gpsimd.dma_start(out=st_t[:, CH:S], in_=st_r[:, CH:S])
                            pre[(0, 'v')] = load_x_chunk(xv, 0, nc.sync, split=True)
                        xt_v = pre[(c, 'v')]
                        # deep prefetch: keep the input stream two chunks
                        # ahead of compute on the dedicated sync queue
                        prefetch(c + 1)
                        prefetch(c + 2)
                        for stl in range(4 * c, 4 * c + 4):
                            vproj_stile(xt_v, stl)
                    # previous chunk's normalize is emitted AFTER this
                    # chunk's proj: its DVE reciprocal then sits behind the
                    # rope work in the DVE queue and can't head-block the
                    # score matmuls' dependencies.
                    if c >= 1:
                        with nc.named_scope(f"norm_c{c - 1}"):
                            normalize_chunk(c - 1, oc_by_chunk[c - 1])
                    def filler(pair, c=c):
                        if c >= 1:
                            wo_stiles(c - 1, which=(2 * pair, 2 * pair + 1))
                    with nc.named_scope(f"att_c{c}"):
                        oc_by_chunk[c] = attention_chunk(c, spair, opsum, filler)
                with nc.named_scope(f"norm_c{NCH - 1}"):
                    normalize_chunk(NCH - 1, oc_by_chunk[NCH - 1])
                with nc.named_scope("wo_c3"):
                    wo_stiles(NCH - 1)

    nc.compile()
    return nc


def _host_tables():
    inv_freq = 1.0 / (10000.0 ** (np.arange(0, DK, 2, dtype=np.float64) / DK))
    pos = np.arange(S, dtype=np.float64)
    fr = pos[:, None] * inv_freq[None, :]          # [S, 32]
    sc8 = 1.0 / math.sqrt(math.sqrt(DK))           # fold 1/sqrt(DK) as sqrt into q and k
    cosT = (np.cos(fr).T * sc8).astype(np.float32)  # [32, S]
    sinT = (np.sin(fr).T * sc8).astype(np.float32)
    C = np.zeros((128, S), np.float32)
    Sg = np.zeros((128, S), np.float32)
    for hh in range(2):
        C[hh * 64:hh * 64 + 32] = cosT
        C[hh * 64 + 32:hh * 64 + 64] = cosT
        Sg[hh * 64:hh * 64 + 32] = -sinT
        Sg[hh * 64 + 32:hh * 64 + 64] = sinT
    # half-swap permutation for two stacked heads (128 rows)
    R = np.zeros((128, 128), np.float32)
    for hh in range(2):
        for j in range(32):
            R[hh * 64 + j, hh * 64 + 32 + j] = 1.0
            R[hh * 64 + 32 + j, hh * 64 + j] = 1.0
    # TRIW[p, y] = 1 iff p <= y - 384; block with offset g uses cols [384-g, 896-g)
    y = np.arange(896)[None, :]
    p = np.arange(128)[:, None]
    TRI = (p <= y - 384).astype(np.float32)
    return C, Sg, R, TRI


_program_cache = {}


def kernel(q, k, v, mask, Wq, bq, Wk, bk, Wv, bv, Wo, bo):
    q = np.asarray(q, np.float32)
    k = np.asarray(k, np.float32)
    v = np.asarray(v, np.float32)
    mask = np.asarray(mask)
    Wq, bq = np.asarray(Wq, np.float32), np.asarray(bq, np.float32)
    Wk, bk = np.asarray(Wk, np.float32), np.asarray(bk, np.float32)
    Wv, bv = np.asarray(Wv, np.float32), np.asarray(bv, np.float32)
    Wo, bo = np.asarray(Wo, np.float32), np.asarray(bo, np.float32)

    causal = np.array_equal(
        np.asarray(mask[0, 0], np.int64), np.tril(np.ones((S, S), np.int64)))
    if not causal or np.any(bq) or np.any(bk):
        return _np_reference_fallback(q, k, v, mask, Wq, bq, Wk, bk, Wv, bv, Wo, bo)

    if "nc" not in _program_cache:
        _program_cache["nc"] = _build_program()
    nc = _program_cache["nc"]

    C, Sg, R, TRI = _host_tables()
    bf = ml_dtypes.bfloat16

    in_maps = []
    for c in range(8):
        b = c // 4
        h0 = 4 * (c % 4)
        sl = slice(h0 * DK, (h0 + 4) * DK)
        in_maps.append({
            "xq": np.ascontiguousarray(q[b].T).astype(bf),
            "xk": np.ascontiguousarray(k[b].T).astype(bf),
            "xv": np.ascontiguousarray(v[b].T).astype(bf),
            "wq": np.ascontiguousarray(Wq[sl].T).astype(bf),
            "wk": np.ascontiguousarray(Wk[sl].T).astype(bf),
            "wv": np.ascontiguousarray(Wv[sl].T).astype(bf),
            "wo": np.ascontiguousarray(Wo[:, sl].T).astype(bf),
            "ct": C.astype(bf),
            "st": Sg.astype(bf),
            "rt": R.astype(bf),
            "tri": TRI.astype(bf),
        })

    res = run_bass_kernel_spmd(nc, in_maps, core_ids=list(range(8)))

    out = np.zeros((B, S, D), np.float32)
    for c in range(8):
        out[c // 4] += res.results[c]["out"].astype(np.float32)
    # bv folds through softmax (rows sum to 1) and Wo; bo direct.
    out += (bv @ Wo.T + bo)[None, None, :]
    return out
